# revision 5
# baseline (speedup 1.0000x reference)
"""AdaptiveEmbedding kernel for 8 TRN2 NeuronCores.

Strategy (moe_routing): host routes tokens to their vocab bucket and splits
each bucket's tokens evenly across 8 cores (data-parallel over tokens,
replicated tables per the sharding hint). Each core gathers only its own
tokens' embedding rows (dma_gather with transpose=True lands features on
partitions, K-chunk contiguous) and runs per-bucket bf16 matmuls against the
projection (PSUM f32 accumulate). Host scatters rows back to token order.

Self-contained: shapes/caps hardcoded; any routing overflow beyond the static
capacities falls back to exact numpy on host.
"""

import numpy as np
import ml_dtypes

BF16 = ml_dtypes.bfloat16

CUT = [0, 20000, 40000, 200000, 267735]
D_EMBS = [1024, 256, 64, 16]
D_PROJ = 1024
NCORES = 8
P = 128
NQ = 4      # SWDGE queues for parallel gather descriptor generation
# sub-gather splits per bucket (dma_gather breaks above ~768 idxs/call)
GSPLIT = [[(0, 256)], [(0, 256)],
          [(0, 512), (512, 512), (1024, 256)], [(0, 640)]]
GQUEUE = {(0, 0): 2, (1, 0): 3, (2, 0): 0, (2, 1): 2, (2, 2): 3, (3, 0): 1}

# per-core per-bucket gather capacities (multiples of 128 for dma_gather)
GC = [256, 256, 1280, 640]
# per-core per-bucket compute widths (tokens actually pushed through matmul)
CW = [192, 192, 1280, 576]
OUT_C = sum(CW)  # 2240
DPAD = [1024, 256, 128, 128]  # feature dims padded to mult of 128
KCH = [d // 128 for d in DPAD]  # [8, 2, 1, 1]
# compacted table row caps for buckets 2/3 (unique rows used by this batch)
TROWS = [20000, 20000, 10240, 4608]

# column offsets
POFF = [0, 8 * 1024, 10 * 1024, 11 * 1024]  # into projs tile, 12*1024 total
IOFF = [0, 16, 32, 112]  # into idx tile (int16 cols, GC/16 each)
IW = 152
TOFF = [0, 192, 384, 1664]  # token column offsets into out slab (cumsum CW)

_CACHE = {}


def _chunks(total, step=512):
    out, t = [], 0
    while t < total:
        w = min(step, total - t)
        out.append((t, w))
        t += w
    return out


def _build():
    import concourse.bacc as bacc
    import concourse.mybir as mybir
    import concourse.tile as tile

    nc = bacc.Bacc("TRN2", target_bir_lowering=False, debug=False,
                   num_devices=NCORES, num_swdge_queues=NQ)
    idxs = nc.declare_dram_parameter("idxs", [P, IW], mybir.dt.int16,
                                     isOutput=False)
    tabs = [
        nc.declare_dram_parameter(f"t{b}", [TROWS[b], DPAD[b]],
                                  mybir.dt.bfloat16, isOutput=False)
        for b in range(4)
    ]
    projs = nc.declare_dram_parameter("projs", [P, 12 * 1024],
                                      mybir.dt.bfloat16, isOutput=False)
    out_t = nc.declare_dram_parameter("out_t", [D_PROJ, OUT_C],
                                      mybir.dt.bfloat16, isOutput=True)

    with tile.TileContext(nc) as tc:
        with (
            tc.tile_pool(name="const", bufs=1) as cpool,
            tc.tile_pool(name="gath", bufs=1) as gpool,
            tc.tile_pool(name="psum", bufs=8, space="PSUM") as ppool,
            tc.tile_pool(name="ostage", bufs=2) as opool,
        ):
            # dummy gather first: forces the mlp ucode library load to start
            # during the NEFF preamble instead of gating the real gathers
            dit = cpool.tile([P, 8], mybir.dt.int16, tag="dummyidx")
            nc.gpsimd.memset(dit[:], 0)
            dg = cpool.tile([P, 1, 128], mybir.dt.bfloat16, tag="dummyg")
            nc.gpsimd.dma_gather(dg[:], tabs[3][:], dit[:], 128,
                                 nc.gpsimd.to_reg(128), DPAD[3],
                                 transpose=True, queue_num=0)

            idx_t = cpool.tile([P, IW], mybir.dt.int16, tag="idx")
            nc.sync.dma_start(out=idx_t[:], in_=idxs[:])
            # per-m projection slices: separate tiles so m-slice 0's matmuls
            # don't wait for the whole 3MB weight stream
            pm = []
            for m in range(8):
                t = cpool.tile([P, 12 * 128], mybir.dt.bfloat16, tag=f"pm{m}")
                nc.sync.dma_start(
                    out=t[:], in_=projs[:, m * 1536:(m + 1) * 1536])
                pm.append(t)

            nregs = {w: nc.gpsimd.to_reg(w)
                     for w in sorted({gw for bs in GSPLIT for (_, gw) in bs})}

            gt = {}
            for b in range(4):
                for s, (g0, gw) in enumerate(GSPLIT[b]):
                    g = gpool.tile([P, KCH[b], gw], mybir.dt.bfloat16,
                                   tag=f"g{b}_{s}")
                    io = IOFF[b] + g0 // 16
                    nc.gpsimd.dma_gather(
                        g[:], tabs[b][:], idx_t[:, io:io + gw // 16],
                        gw, nregs[gw], DPAD[b], transpose=True,
                        queue_num=GQUEUE[(b, s)],
                    )
                    gt[(b, s)] = g

            mcount = 0
            for m in range(8):
                mrow = opool.tile([P, OUT_C], mybir.dt.bfloat16, tag="mrow")
                for b in range(4):
                    for s, (g0, gw) in enumerate(GSPLIT[b]):
                        cwid = min(gw, max(0, CW[b] - g0))
                        for (t0, w) in _chunks(cwid):
                            ps = ppool.tile([P, 512], mybir.dt.float32,
                                            tag="ps")
                            for kk in range(KCH[b]):
                                c0 = (POFF[b] // 1024 + kk) * 128
                                nc.tensor.matmul(
                                    ps[:, :w],
                                    pm[m][:, c0:c0 + 128],
                                    gt[(b, s)][:, kk, t0:t0 + w],
                                    start=(kk == 0),
                                    stop=(kk == KCH[b] - 1),
                                )
                            col = TOFF[b] + g0 + t0
                            dst = mrow[:, col:col + w]
                            if mcount % 2 == 0:
                                nc.vector.tensor_copy(dst, ps[:, :w])
                            else:
                                nc.scalar.copy(dst, ps[:, :w])
                            mcount += 1
                nc.sync.dma_start(
                    out=out_t[m * 128:(m + 1) * 128, :], in_=mrow[:],
                )
    nc.compile()
    return nc


def _route(flat):
    """Split tokens by bucket; remap buckets 2/3 through compacted tables."""
    b_of = np.searchsorted(np.asarray(CUT[1:-1]), flat, side="right")
    toks, locs, uniq = [], [], [None, None, None, None]
    fallback = []  # (token_id, bucket, local_row)
    for b in range(4):
        tb = np.nonzero(b_of == b)[0]
        lb = (flat[tb] - CUT[b]).astype(np.int64)
        if b >= 2:
            u, inv = np.unique(lb, return_inverse=True)
            if len(u) > TROWS[b]:
                keep = inv < TROWS[b]
                for t, r in zip(tb[~keep], lb[~keep]):
                    fallback.append((int(t), b, int(r)))
                tb, inv = tb[keep], inv[keep]
                u = u[:TROWS[b]]
            uniq[b] = u
            lb = inv
        toks.append(tb)
        locs.append(lb)
    return toks, locs, uniq, fallback


def kernel(inp, emb0, emb1, emb2, emb3, proj0, proj1, proj2, proj3):
    from concourse.bass_utils import run_bass_kernel_spmd

    embs = [np.asarray(emb0), np.asarray(emb1), np.asarray(emb2),
            np.asarray(emb3)]
    projs_in = [np.asarray(proj0), np.asarray(proj1), np.asarray(proj2),
                np.asarray(proj3)]
    inp = np.asarray(inp)
    flat = inp.reshape(-1).astype(np.int64)
    N = flat.shape[0]

    toks, locs, uniq, fallback = _route(flat)

    # --- tables (bf16, feature-padded; buckets 2/3 compacted to used rows)
    tabs = []
    for b in range(4):
        if b < 2:
            tabs.append(np.ascontiguousarray(embs[b].astype(BF16)))
        else:
            t = np.zeros((TROWS[b], DPAD[b]), BF16)
            u = uniq[b]
            t[:len(u), :D_EMBS[b]] = embs[b][u].astype(BF16)
            tabs.append(t)

    # --- projection layout: [p, m*1536 + j*128 + c] = proj_b[m*128+c, kk*128+p]
    # where j = POFF[b]//1024 + kk (12 (b,kk) pairs)
    pj = np.zeros((P, 12 * 1024), BF16)
    for b in range(4):
        pt = projs_in[b].T.astype(BF16)  # [d_b, 1024]
        for kk in range(KCH[b]):
            rows = pt[kk * 128:(kk + 1) * 128]  # [<=128, 1024]
            j = POFF[b] // 1024 + kk
            for m in range(8):
                pj[:rows.shape[0], m * 1536 + j * 128:m * 1536 + (j + 1) * 128] = \
                    rows[:, m * 128:(m + 1) * 128]

    # --- per-core idx tiles + scatter bookkeeping
    in_maps = []
    core_tok = []  # [core][bucket] -> token ids computed on device
    for c in range(NCORES):
        it = np.zeros((P, IW), np.int16)
        ct = []
        for b in range(4):
            tb = toks[b][c::NCORES]
            lb = locs[b][c::NCORES]
            if len(tb) > CW[b]:
                for t, r in zip(tb[CW[b]:], lb[CW[b]:]):
                    if b >= 2:
                        r = int(uniq[b][r])
                    fallback.append((int(t), b, int(r)))
                tb, lb = tb[:CW[b]], lb[:CW[b]]
            ct.append(tb)
            idx = np.zeros(GC[b], np.int16)
            idx[:len(lb)] = lb.astype(np.int16)
            wrapped = idx.reshape(GC[b] // 16, 16).T  # [16, GC/16]
            it[:, IOFF[b]:IOFF[b] + GC[b] // 16] = np.tile(wrapped, (8, 1))
        core_tok.append(ct)
        in_maps.append({
            "idxs": it,
            "t0": tabs[0], "t1": tabs[1], "t2": tabs[2], "t3": tabs[3],
            "projs": pj,
        })

    if "nc" not in _CACHE:
        _CACHE["nc"] = _build()
    nc = _CACHE["nc"]

    res = run_bass_kernel_spmd(nc, in_maps, core_ids=list(range(NCORES)))
    _CACHE["last_result"] = res

    # --- scatter back
    final = np.zeros((N, D_PROJ), np.float32)
    for c in range(NCORES):
        slab = res.results[c]["out_t"].astype(np.float32)  # [1024, OUT_C]
        for b in range(4):
            tb = core_tok[c][b]
            n = len(tb)
            if n:
                final[tb] = slab[:, TOFF[b]:TOFF[b] + n].T

    for (t, b, r) in fallback:
        final[t] = embs[b][r].astype(np.float32) @ projs_in[b].T

    return final.reshape(*inp.shape, D_PROJ)


# revision 7
# speedup vs baseline: 1.1467x; 1.1467x over previous
"""AdaptiveEmbedding kernel for 8 TRN2 NeuronCores.

Strategy (moe_routing): host routes tokens to their vocab bucket and splits
each bucket's tokens evenly across 8 cores (data-parallel over tokens,
replicated tables per the sharding hint). Each core gathers only its own
tokens' embedding rows (dma_gather with transpose=True lands features on
partitions, K-chunk contiguous) and runs per-bucket bf16 matmuls against the
projection (PSUM f32 accumulate). Host scatters rows back to token order.

Self-contained: shapes/caps hardcoded; any routing overflow beyond the static
capacities falls back to exact numpy on host.
"""

import numpy as np
import ml_dtypes

BF16 = ml_dtypes.bfloat16

CUT = [0, 20000, 40000, 200000, 267735]
D_EMBS = [1024, 256, 64, 16]
D_PROJ = 1024
NCORES = 8
P = 128
NQ = 4      # SWDGE queues for parallel gather descriptor generation
# sub-gather splits per bucket (dma_gather breaks above ~768 idxs/call)
GSPLIT = [[(0, 256)], [(0, 256)],
          [(0, 512), (512, 512), (1024, 256)], [(0, 640)]]
GQUEUE = {(0, 0): 2, (1, 0): 3, (2, 0): 0, (2, 1): 2, (2, 2): 3, (3, 0): 1}

# per-core per-bucket gather capacities (multiples of 128 for dma_gather)
GC = [256, 256, 1280, 640]
# per-core per-bucket compute widths (tokens actually pushed through matmul)
CW = [192, 192, 1280, 576]
OUT_C = sum(CW)  # 2240
DPAD = [1024, 256, 128, 128]  # feature dims padded to mult of 128
KCH = [d // 128 for d in DPAD]  # [8, 2, 1, 1]
# compacted table row caps for buckets 2/3 (unique rows used by this batch)
TROWS = [20000, 20000, 10240, 4608]

# column offsets
POFF = [0, 8 * 1024, 10 * 1024, 11 * 1024]  # into projs tile, 12*1024 total
IOFF = [0, 16, 32, 112]  # into idx tile (int16 cols, GC/16 each)
IW = 152
TOFF = [0, 192, 384, 1664]  # token column offsets into out slab (cumsum CW)

_CACHE = {}


def _chunks(total, step=512):
    out, t = [], 0
    while t < total:
        w = min(step, total - t)
        out.append((t, w))
        t += w
    return out


def _build():
    import concourse.bacc as bacc
    import concourse.mybir as mybir
    import concourse.tile as tile
    from concourse.tile import add_dep_helper

    nc = bacc.Bacc("TRN2", target_bir_lowering=False, debug=False,
                   num_devices=NCORES, num_swdge_queues=NQ)
    idxs = nc.declare_dram_parameter("idxs", [P, IW], mybir.dt.int16,
                                     isOutput=False)
    tabs = [
        nc.declare_dram_parameter(f"t{b}", [TROWS[b], DPAD[b]],
                                  mybir.dt.bfloat16, isOutput=False)
        for b in range(4)
    ]
    projs = nc.declare_dram_parameter("projs", [P, 12 * 1024],
                                      mybir.dt.bfloat16, isOutput=False)
    out_t = nc.declare_dram_parameter("out_t", [D_PROJ, OUT_C],
                                      mybir.dt.bfloat16, isOutput=True)

    with tile.TileContext(nc) as tc:
        with (
            tc.tile_pool(name="const", bufs=1) as cpool,
            tc.tile_pool(name="gath", bufs=1) as gpool,
            tc.tile_pool(name="psum", bufs=8, space="PSUM") as ppool,
            tc.tile_pool(name="ostage", bufs=2) as opool,
        ):
            # dummy gather first: forces the mlp ucode library load to start
            # during the NEFF preamble instead of gating the real gathers
            dit = cpool.tile([P, 8], mybir.dt.int16, tag="dummyidx")
            nc.gpsimd.memset(dit[:], 0)
            dg = cpool.tile([P, 1, 128], mybir.dt.bfloat16, tag="dummyg")
            dummy_inst = nc.gpsimd.dma_gather(
                dg[:], tabs[3][:], dit[:], 128, nc.gpsimd.to_reg(128),
                DPAD[3], transpose=True, queue_num=0)

            idx_t = cpool.tile([P, IW], mybir.dt.int16, tag="idx")
            nc.sync.dma_start(out=idx_t[:], in_=idxs[:])
            # per-m projection slices: separate tiles so m-slice 0's matmuls
            # don't wait for the whole 3MB weight stream
            # the 3MB projection stream must not start before the gather
            # ucode library fetch completes (it would monopolize all 16 DMA
            # engines and push the gathers out by ~10us)
            pm = []
            for m in range(8):
                t = cpool.tile([P, 12 * 128], mybir.dt.bfloat16, tag=f"pm{m}")
                di = nc.sync.dma_start(
                    out=t[:], in_=projs[:, m * 1536:(m + 1) * 1536])
                add_dep_helper(dummy_inst.ins, di.ins, sync=True,
                               reason="projs stream after gather lib load")
                pm.append(t)

            nregs = {w: nc.gpsimd.to_reg(w)
                     for w in sorted({gw for bs in GSPLIT for (_, gw) in bs})}

            gt = {}
            for b in range(4):
                for s, (g0, gw) in enumerate(GSPLIT[b]):
                    g = gpool.tile([P, KCH[b], gw], mybir.dt.bfloat16,
                                   tag=f"g{b}_{s}")
                    io = IOFF[b] + g0 // 16
                    nc.gpsimd.dma_gather(
                        g[:], tabs[b][:], idx_t[:, io:io + gw // 16],
                        gw, nregs[gw], DPAD[b], transpose=True,
                        queue_num=GQUEUE[(b, s)],
                    )
                    gt[(b, s)] = g

            mcount = 0
            for m in range(8):
                mrow = opool.tile([P, OUT_C], mybir.dt.bfloat16, tag="mrow")
                for b in range(4):
                    for s, (g0, gw) in enumerate(GSPLIT[b]):
                        cwid = min(gw, max(0, CW[b] - g0))
                        for (t0, w) in _chunks(cwid):
                            ps = ppool.tile([P, 512], mybir.dt.float32,
                                            tag="ps")
                            for kk in range(KCH[b]):
                                c0 = (POFF[b] // 1024 + kk) * 128
                                nc.tensor.matmul(
                                    ps[:, :w],
                                    pm[m][:, c0:c0 + 128],
                                    gt[(b, s)][:, kk, t0:t0 + w],
                                    start=(kk == 0),
                                    stop=(kk == KCH[b] - 1),
                                )
                            col = TOFF[b] + g0 + t0
                            dst = mrow[:, col:col + w]
                            if mcount % 2 == 0:
                                nc.vector.tensor_copy(dst, ps[:, :w])
                            else:
                                nc.scalar.copy(dst, ps[:, :w])
                            mcount += 1
                nc.sync.dma_start(
                    out=out_t[m * 128:(m + 1) * 128, :], in_=mrow[:],
                )
    nc.compile()
    return nc


def _route(flat):
    """Split tokens by bucket; remap buckets 2/3 through compacted tables."""
    b_of = np.searchsorted(np.asarray(CUT[1:-1]), flat, side="right")
    toks, locs, uniq = [], [], [None, None, None, None]
    fallback = []  # (token_id, bucket, local_row)
    for b in range(4):
        tb = np.nonzero(b_of == b)[0]
        lb = (flat[tb] - CUT[b]).astype(np.int64)
        if b >= 2:
            u, inv = np.unique(lb, return_inverse=True)
            if len(u) > TROWS[b]:
                keep = inv < TROWS[b]
                for t, r in zip(tb[~keep], lb[~keep]):
                    fallback.append((int(t), b, int(r)))
                tb, inv = tb[keep], inv[keep]
                u = u[:TROWS[b]]
            uniq[b] = u
            lb = inv
        toks.append(tb)
        locs.append(lb)
    return toks, locs, uniq, fallback


def kernel(inp, emb0, emb1, emb2, emb3, proj0, proj1, proj2, proj3):
    from concourse.bass_utils import run_bass_kernel_spmd

    embs = [np.asarray(emb0), np.asarray(emb1), np.asarray(emb2),
            np.asarray(emb3)]
    projs_in = [np.asarray(proj0), np.asarray(proj1), np.asarray(proj2),
                np.asarray(proj3)]
    inp = np.asarray(inp)
    flat = inp.reshape(-1).astype(np.int64)
    N = flat.shape[0]

    toks, locs, uniq, fallback = _route(flat)

    # --- tables (bf16, feature-padded; buckets 2/3 compacted to used rows)
    tabs = []
    for b in range(4):
        if b < 2:
            tabs.append(np.ascontiguousarray(embs[b].astype(BF16)))
        else:
            t = np.zeros((TROWS[b], DPAD[b]), BF16)
            u = uniq[b]
            t[:len(u), :D_EMBS[b]] = embs[b][u].astype(BF16)
            tabs.append(t)

    # --- projection layout: [p, m*1536 + j*128 + c] = proj_b[m*128+c, kk*128+p]
    # where j = POFF[b]//1024 + kk (12 (b,kk) pairs)
    pj = np.zeros((P, 12 * 1024), BF16)
    for b in range(4):
        pt = projs_in[b].T.astype(BF16)  # [d_b, 1024]
        for kk in range(KCH[b]):
            rows = pt[kk * 128:(kk + 1) * 128]  # [<=128, 1024]
            j = POFF[b] // 1024 + kk
            for m in range(8):
                pj[:rows.shape[0], m * 1536 + j * 128:m * 1536 + (j + 1) * 128] = \
                    rows[:, m * 128:(m + 1) * 128]

    # --- per-core idx tiles + scatter bookkeeping
    in_maps = []
    core_tok = []  # [core][bucket] -> token ids computed on device
    for c in range(NCORES):
        it = np.zeros((P, IW), np.int16)
        ct = []
        for b in range(4):
            tb = toks[b][c::NCORES]
            lb = locs[b][c::NCORES]
            if len(tb) > CW[b]:
                for t, r in zip(tb[CW[b]:], lb[CW[b]:]):
                    if b >= 2:
                        r = int(uniq[b][r])
                    fallback.append((int(t), b, int(r)))
                tb, lb = tb[:CW[b]], lb[:CW[b]]
            ct.append(tb)
            idx = np.zeros(GC[b], np.int16)
            idx[:len(lb)] = lb.astype(np.int16)
            wrapped = idx.reshape(GC[b] // 16, 16).T  # [16, GC/16]
            it[:, IOFF[b]:IOFF[b] + GC[b] // 16] = np.tile(wrapped, (8, 1))
        core_tok.append(ct)
        in_maps.append({
            "idxs": it,
            "t0": tabs[0], "t1": tabs[1], "t2": tabs[2], "t3": tabs[3],
            "projs": pj,
        })

    if "nc" not in _CACHE:
        _CACHE["nc"] = _build()
    nc = _CACHE["nc"]

    res = run_bass_kernel_spmd(nc, in_maps, core_ids=list(range(NCORES)))
    _CACHE["last_result"] = res

    # --- scatter back
    final = np.zeros((N, D_PROJ), np.float32)
    for c in range(NCORES):
        slab = res.results[c]["out_t"].astype(np.float32)  # [1024, OUT_C]
        for b in range(4):
            tb = core_tok[c][b]
            n = len(tb)
            if n:
                final[tb] = slab[:, TOFF[b]:TOFF[b] + n].T

    for (t, b, r) in fallback:
        final[t] = embs[b][r].astype(np.float32) @ projs_in[b].T

    return final.reshape(*inp.shape, D_PROJ)
